# revision 2
# baseline (speedup 1.0000x reference)
"""Trainium2 Bass kernel for BilinearInteraction, v2.

out[b, p, :] = (x[b, i_p, :] @ W[p]) * x[b, j_p, :]  for pairs p=(i,j), i<j
B=4096, F=32, D=64, P=496.

Strategy (vs fp32 v1 at 218us):
 - fp16 output stores (rel err ~6e-3 vs 2e-2 gate): store traffic 65->32.5MB
   per core; HBM floor ~(32.5+12.4)/358 = 125us.
 - Matmul in bf16 hi/lo 3-pass (xH@WH + xH@WL + xL@WH accumulated in fp32
   PSUM; measured back-to-back bf16 MMs stream at 1 cyc/col). Even/odd
   feature blocks go to disjoint PE row halves and their passes interleave.
 - x transposed on PE in fp32 (16/tile), hi/lo split in transposed form:
   ACT copies ptr->xtH (bf16 cast), DVE subtracts ->xtL.
 - The vj elementwise multiply is split across three lanes to balance
   engines: D = DVE mul psum*x fp32->fp16 direct (1.042 ns/col);
   A = ACT copy psum->fp16 tmp (0.833) + DVE fp16 mul (0.521, 2x mode);
   G = ACT copy + GPSIMD fp16 mul (~2 ns/col, otherwise idle engine).
 - PSUM: 3x [128,1024] macro tiles (2 banks each) + 2x [128,512] ptr banks;
   per-piece allocation, consumers emitted the moment a piece's MMs end,
   so PE production and DVE/ACT/GPS consumption overlap (v2.0's duo
   lockstep serialized them and ran 299us).
"""

import numpy as np

B, F, D = 4096, 32, 64
P = F * (F - 1) // 2            # 496
NCORES = 8
BLOC = B // NCORES              # 512
BT = 128                        # batch tile rows
NBT = BLOC // BT                # 4
NT = F // 2                     # fp32 transposes per batch tile (16)
TOTCOL = P * D                  # 31744
WIN = 6144                      # output SBUF window columns
MACRO = 1024                    # PSUM macro tile columns (2 banks)
MM = 512                        # max matmul free dim (one PSUM bank)
WCHUNK = 4096                   # W load chunk columns


def _p0(i):
    return i * (F - 1) - i * (i - 1) // 2


def _blocks():
    """(i, gs, ge, parity_offset) per feature block, in i order."""
    out = []
    off = {0: 0, 1: 0}
    for i in range(F - 1):
        gs = _p0(i) * D
        w = (F - 1 - i) * D
        out.append((i, gs, gs + w, off[i % 2]))
        off[i % 2] += w
    return out


BLOCKS = _blocks()
W_EVEN_COLS = sum(ge - gs for i, gs, ge, _ in BLOCKS if i % 2 == 0)   # 16384
W_ODD_COLS = sum(ge - gs for i, gs, ge, _ in BLOCKS if i % 2 == 1)    # 15360


def _pieces_of_block(block):
    """Pieces <= MACRO cols, never crossing a WIN line (6144%1024==0 so
    window lines are also MACRO lines)."""
    i, gs, ge, po = block
    out = []
    c = gs
    while c < ge:
        c1 = min(ge, c + MACRO, (c // WIN + 1) * WIN)
        out.append((i, c, c1, po + (c - gs)))
        c = c1
    return out


def _chunks_of_piece(piece):
    """Matmul chunks <= MM cols on the piece-local 512 grid (PSUM banks)."""
    i, c0, c1, wo = piece
    out = []
    c = c0
    while c < c1:
        local = c - c0
        c2 = min(c1, c0 + (local // MM + 1) * MM)
        out.append((c, c2, wo + (c - c0)))
        c = c2
    return out


def _group_pairs():
    pairs = []
    for k in range(0, F - 1, 2):
        a = _pieces_of_block(BLOCKS[k])
        b = _pieces_of_block(BLOCKS[k + 1]) if k + 1 < F - 1 else []
        pairs.append((a, b))
    return pairs


GROUP_PAIRS = _group_pairs()
ALL_PIECES = [p for a, b in GROUP_PAIRS for p in a + b]


def _duo_stream():
    """Simulate the MM emission round-robin; return the piece completion
    order (determines consume op order and window store points)."""
    order = []
    for a, b in GROUP_PAIRS:
        for k in range(max(len(a), len(b))):
            duo = []
            if k < len(a):
                duo.append(a[k])
            if k < len(b):
                duo.append(b[k])
            counts = [3 * len(_chunks_of_piece(p)) for p in duo]
            done = [0] * len(duo)
            while any(done[m] < counts[m] for m in range(len(duo))):
                for m in range(len(duo)):
                    if done[m] < counts[m]:
                        done[m] += 1
                        if done[m] == counts[m]:
                            order.append(duo[m])
    return order


CONSUME_ORDER = _duo_stream()


def _lane_plan():
    """Lane per piece (greedy, measured costs): D = DVE direct mul;
    A = ACT copy + DVE fp16 mul; G = ACT copy + GPSIMD fp16 mul.
    Soft-alternate the PSUM-reader engine (DVE for D, ACT for A/G) so
    adjacent pieces' evacuations overlap; penalize repeating a reader."""
    actns = 2800.0 + 1000.0  # xt copies + half the xj16 copies
    dvens = 2800.0           # xt subs
    gpsns = 2500.0 + 4200.0  # W dma descriptors + half the xj16 copies
    lanes = {}
    prev_reader = None
    for (i, c0, c1, wo) in CONSUME_ORDER:
        n = c1 - c0
        cA_act = 0.833 * n + 264
        cA_dve = 0.521 * n + 159
        cG_gps = 2.07 * n + 250
        cD_dve = 1.042 * n + 157
        PEN = 450.0
        opts = [
            ('D', 'V', max(actns, dvens + cD_dve, gpsns)
             + (PEN if prev_reader == 'V' else 0)),
            ('A', 'S', max(actns + cA_act, dvens + cA_dve, gpsns)
             + (PEN if prev_reader == 'S' else 0)),
        ]
        if n >= 256:
            opts.append(('G', 'S', max(actns + cA_act, dvens, gpsns + cG_gps)
                         + (PEN if prev_reader == 'S' else 0)))
        lane, reader, _ = min(opts, key=lambda o: o[2])
        lanes[(i, c0)] = lane
        prev_reader = reader
        if lane == 'D':
            dvens += cD_dve
        elif lane == 'A':
            actns += cA_act
            dvens += cA_dve
        else:
            actns += cA_act
            gpsns += cG_gps
    return lanes, actns, dvens, gpsns


LANES, _ACT_NS, _DVE_NS, _GPS_NS = _lane_plan()


def _win_emit_map():
    emit = {}
    for (i, c0, c1, wo) in CONSUME_ORDER:
        emit[c0 // WIN] = (i, c0)
    by_piece = {}
    for w, key in emit.items():
        by_piece.setdefault(key, []).append(w)
    return by_piece


WIN_EMIT = _win_emit_map()
NWIN = (TOTCOL + WIN - 1) // WIN


def build_bass():
    import concourse.bacc as bacc
    import concourse.mybir as mybir
    from concourse import tile

    fp32 = mybir.dt.float32
    bf16 = mybir.dt.bfloat16
    fp16 = mybir.dt.float16
    nc = bacc.Bacc("TRN2", target_bir_lowering=False, debug=False)

    x_dram = nc.dram_tensor("x", [BLOC, F * D], fp32, kind="ExternalInput")
    whe_dram = nc.dram_tensor("wh_even", [D, W_EVEN_COLS], bf16, kind="ExternalInput")
    who_dram = nc.dram_tensor("wh_odd", [D, W_ODD_COLS], bf16, kind="ExternalInput")
    wle_dram = nc.dram_tensor("wl_even", [D, W_EVEN_COLS], bf16, kind="ExternalInput")
    wlo_dram = nc.dram_tensor("wl_odd", [D, W_ODD_COLS], bf16, kind="ExternalInput")
    id_dram = nc.dram_tensor("ident", [BT, BT], fp32, kind="ExternalInput")
    out_dram = nc.dram_tensor("out", [BLOC, TOTCOL], fp16, kind="ExternalOutput")

    with tile.TileContext(nc) as tc:
        with (
            tc.tile_pool(name="const", bufs=1) as const_pool,
            tc.tile_pool(name="x", bufs=2) as x_pool,
            tc.tile_pool(name="xj", bufs=2) as xj_pool,
            tc.tile_pool(name="xth", bufs=2) as xth_pool,
            tc.tile_pool(name="xtl", bufs=2) as xtl_pool,
            tc.tile_pool(name="tmp", bufs=3) as tmp_pool,
            tc.tile_pool(name="outw", bufs=3) as out_pool,
            tc.tile_pool(name="pmm", bufs=4, space="PSUM") as pmm_pool,
        ):
            ident = const_pool.tile([BT, BT], fp32, tag="ident")
            nc.sync.dma_start(ident[:], id_dram[:])

            x_tiles = [None] * NBT
            xj_tiles = [None] * NBT
            xth_tiles = [None] * NBT
            xtl_tiles = [None] * NBT

            x_tiles[0] = x_pool.tile([BT, F * D], fp32, tag="x", name="x_0")
            nc.sync.dma_start(x_tiles[0][:], x_dram[0:BT, :])

            # W hi/lo loads on SWDGE (gpsimd): coalesced column chunks,
            # interleaved across the 4 streams in consumption order.
            wh_sb = const_pool.tile([128, W_EVEN_COLS], bf16, tag="wh")
            wl_sb = const_pool.tile([128, W_EVEN_COLS], bf16, tag="wl")
            he, ho = slice(0, 64), slice(64, 128)
            for c in range(0, W_EVEN_COLS, WCHUNK):
                ce = min(W_EVEN_COLS, c + WCHUNK)
                co = min(W_ODD_COLS, c + WCHUNK)
                nc.gpsimd.dma_start(wh_sb[he, c:ce], whe_dram[:, c:ce])
                if c < W_ODD_COLS:
                    nc.gpsimd.dma_start(wh_sb[ho, c:co], who_dram[:, c:co])
                nc.gpsimd.dma_start(wl_sb[he, c:ce], wle_dram[:, c:ce])
                if c < W_ODD_COLS:
                    nc.gpsimd.dma_start(wl_sb[ho, c:co], wlo_dram[:, c:co])

            def alloc_xt(bt):
                xth_tiles[bt] = xth_pool.tile([BT, NT * BT], bf16, tag="xth",
                                              name=f"xth_{bt}")
                xtl_tiles[bt] = xtl_pool.tile([BT, NT * BT], bf16, tag="xtl",
                                              name=f"xtl_{bt}")

            def emit_xj(bt):
                xj_tiles[bt] = xj_pool.tile([BT, F * D], fp16, tag="xj",
                                            name=f"xj_{bt}")
                eng = nc.scalar if bt % 2 == 0 else nc.gpsimd
                if bt % 2 == 0:
                    eng.copy(xj_tiles[bt][:], x_tiles[bt][:])
                else:
                    eng.tensor_copy(xj_tiles[bt][:], x_tiles[bt][:])

            def emit_transpose_batch(bt, q):
                # transposes borrow a pmm-pool slot (first bank of it)
                ptr = pmm_pool.tile([BT, MACRO], fp32, tag="pmm",
                                    name=f"ptr_{bt}_{q}")
                for s in range(4):
                    t = 4 * q + s
                    nc.tensor.transpose(
                        ptr[:, s * BT:(s + 1) * BT],
                        x_tiles[bt][:, t * BT:(t + 1) * BT], ident[:]
                    )
                cols = slice(4 * q * BT, 4 * (q + 1) * BT)
                nc.scalar.copy(xth_tiles[bt][:, cols], ptr[:, 0:4 * BT])
                nc.vector.tensor_sub(
                    xtl_tiles[bt][:, cols], ptr[:, 0:4 * BT],
                    xth_tiles[bt][:, cols]
                )

            for bt in range(NBT):
                rows = slice(bt * BT, (bt + 1) * BT)
                if bt == 0:
                    alloc_xt(0)
                    for q in range(4):
                        emit_transpose_batch(0, q)
                    emit_xj(0)
                nxt = bt + 1
                if nxt < NBT:
                    x_tiles[nxt] = x_pool.tile([BT, F * D], fp32, tag="x",
                                               name=f"x_{nxt}")
                    nc.scalar.dma_start(
                        x_tiles[nxt][:], x_dram[nxt * BT:(nxt + 1) * BT, :]
                    )
                    alloc_xt(nxt)

                win_tiles = {}
                x_sb = x_tiles[bt]
                xj_sb = xj_tiles[bt]
                xth_sb = xth_tiles[bt]
                xtl_sb = xtl_tiles[bt]

                def get_win(w):
                    if w not in win_tiles:
                        win_tiles[w] = out_pool.tile(
                            [BT, WIN], fp16, tag="win", name=f"win_{bt}_{w}"
                        )
                    return win_tiles[w]

                def consume_piece(piece, pmm):
                    (i, c0, c1, wo) = piece
                    n = c1 - c0
                    w = c0 // WIN
                    wt = get_win(w)
                    l0 = c0 - w * WIN
                    xoff = (i + 1) * D + (c0 - _p0(i) * D)
                    lane = LANES[(i, c0)]
                    if lane == 'D':
                        nc.vector.tensor_mul(
                            wt[:, l0:l0 + n], pmm[:, 0:n],
                            x_sb[:, xoff:xoff + n],
                        )
                    else:
                        tmp = tmp_pool.tile([BT, MACRO], fp16, tag="tmp",
                                            name=f"tmp_{bt}_{i}_{c0}")
                        nc.scalar.copy(tmp[:, 0:n], pmm[:, 0:n])
                        eng = nc.vector if lane == 'A' else nc.gpsimd
                        eng.tensor_mul(
                            wt[:, l0:l0 + n], tmp[:, 0:n],
                            xj_sb[:, xoff:xoff + n],
                        )
                    for wi in WIN_EMIT.get((i, c0), ()):
                        wt2 = win_tiles[wi]
                        c0w = wi * WIN
                        wd = min(WIN, TOTCOL - c0w)
                        nc.sync.dma_start(
                            out_dram[rows, c0w:c0w + wd], wt2[:, 0:wd]
                        )

                def piece_mm_ops(piece, pmm):
                    (i, c0, c1, wo) = piece
                    par = i % 2
                    prows = he if par == 0 else ho
                    tpos = (0, 0) if par == 0 else (64, 0)
                    t = i // 2
                    lhH = xth_sb[prows, t * BT:(t + 1) * BT]
                    lhL = xtl_sb[prows, t * BT:(t + 1) * BT]
                    ops = []
                    for (g0, g1, woff) in _chunks_of_piece(piece):
                        csz = g1 - g0
                        lo = g0 - c0
                        dst = pmm[:, lo:lo + csz]
                        wh = wh_sb[prows, woff:woff + csz]
                        wl = wl_sb[prows, woff:woff + csz]
                        ops.append((dst, lhH, wh, True, False, tpos))
                        ops.append((dst, lhH, wl, False, False, tpos))
                        ops.append((dst, lhL, wh, False, True, tpos))
                    return ops

                tq = 0
                for gidx, (apieces, bpieces) in enumerate(GROUP_PAIRS):
                    for k in range(max(len(apieces), len(bpieces))):
                        duo = []
                        if k < len(apieces):
                            duo.append(apieces[k])
                        if k < len(bpieces):
                            duo.append(bpieces[k])
                        pend = []
                        mms = []
                        for piece in duo:
                            pmm = pmm_pool.tile(
                                [BT, MACRO], mybir.dt.float32, tag="pmm",
                                name=f"pmm_{bt}_{piece[0]}_{piece[1]}")
                            pend.append((piece, pmm))
                            mms.append(piece_mm_ops(piece, pmm))
                        done = [0] * len(mms)
                        while any(done[m] < len(mms[m]) for m in range(len(mms))):
                            for m in range(len(mms)):
                                if done[m] < len(mms[m]):
                                    dst, lh, rh, st, sp, tp = mms[m][done[m]]
                                    nc.tensor.matmul(dst, lh, rh, start=st,
                                                     stop=sp, tile_position=tp)
                                    done[m] += 1
                                    if done[m] == len(mms[m]):
                                        consume_piece(*pend[m])
                    if nxt < NBT and gidx in (2, 5, 8, 11):
                        emit_transpose_batch(nxt, tq)
                        tq += 1
                        if gidx == 11:
                            emit_xj(nxt)

    nc.compile()
    return nc


_CACHE = {}


def _get_nc():
    if "nc" not in _CACHE:
        _CACHE["nc"] = build_bass()
    return _CACHE["nc"]


def _host_w_split(W):
    import ml_dtypes
    Wt = np.ascontiguousarray(
        np.asarray(W, dtype=np.float32).transpose(1, 0, 2)
    ).reshape(D, TOTCOL)
    WH = Wt.astype(ml_dtypes.bfloat16)
    WL = (Wt - WH.astype(np.float32)).astype(ml_dtypes.bfloat16)
    wh_even = np.ascontiguousarray(np.concatenate(
        [WH[:, gs:ge] for i, gs, ge, _ in BLOCKS if i % 2 == 0], axis=1))
    wh_odd = np.ascontiguousarray(np.concatenate(
        [WH[:, gs:ge] for i, gs, ge, _ in BLOCKS if i % 2 == 1], axis=1))
    wl_even = np.ascontiguousarray(np.concatenate(
        [WL[:, gs:ge] for i, gs, ge, _ in BLOCKS if i % 2 == 0], axis=1))
    wl_odd = np.ascontiguousarray(np.concatenate(
        [WL[:, gs:ge] for i, gs, ge, _ in BLOCKS if i % 2 == 1], axis=1))
    return wh_even, wh_odd, wl_even, wl_odd


def _in_maps(inputs):
    x = np.asarray(inputs["inputs"], dtype=np.float32).reshape(B, F * D)
    wh_even, wh_odd, wl_even, wl_odd = _host_w_split(inputs["W"])
    ident = np.eye(BT, dtype=np.float32)
    return [
        {
            "x": np.ascontiguousarray(x[c * BLOC:(c + 1) * BLOC]),
            "wh_even": wh_even, "wh_odd": wh_odd,
            "wl_even": wl_even, "wl_odd": wl_odd,
            "ident": ident,
        }
        for c in range(NCORES)
    ]


def kernel(inputs, W):
    from concourse import bass_utils

    in_maps = _in_maps({"inputs": inputs, "W": W})
    nc = _get_nc()
    res = bass_utils.run_bass_kernel_spmd(nc, in_maps, core_ids=list(range(NCORES)))
    out = np.concatenate(
        [np.asarray(res.results[c]["out"]).astype(np.float32)
         for c in range(NCORES)], axis=0)
    return out.reshape(B, P, D)
